# revision 10
# baseline (speedup 1.0000x reference)
"""Trainium2 Bass kernel for causal self-attention (nn_Casualselfatt).

Reference computes (B=2, S=2048, E=1024, H=16, D=64, fp32):
    qkv = x @ W_qkv + b_qkv ; q,k,v = split(qkv)
    q = q.reshape(B, H, S, D)   # NOTE: raw reshape, no transpose.
    ...causal softmax attention per (b,h)...
    out = res @ W_proj + b_proj

The raw reshape means head h of batch b attends over the [S, D] reshape of
rows [128h, 128h+128) of q/k/v[b].  Sharding: 32 (b,h) pairs -> 4 heads of
one batch per core (core c: b=c//4, heads 4*(c%4)..+4).  Each core computes
a partial projection output; the host sums 4 partials per batch.

On-chip: scores are built transposed ([k-part, q-free]) so the softmax
denominator rides an appended ones-column through the AV matmul.  QKV runs
in bf16 (fp32 accumulate); scores run in float32r; the post-softmax path is
bf16.  x arrives pre-transposed from the host (bf16), W_qkv is SBUF-resident
(streamed once), and QKV psum->SBUF copies are pair-merged and split across
DVE/ACT.  PSUM: sc pool 4 banks + shared 1-bank work pool (qkv ps / av /
vtrans vp / proj pp) x4 bufs so attention groups double-buffer across g.
Softmax reciprocal uses the fast approx DVE op (~5x cheaper).
"""

import numpy as np
import ml_dtypes

import concourse.bass as bass
import concourse.tile as tile
from concourse import bacc, mybir
import concourse.bass_utils as bass_utils

F32 = mybir.dt.float32
F32R = mybir.dt.float32r
BF16 = mybir.dt.bfloat16

B, S, E = 2, 2048, 1024
H, D = 16, 64
N_CORES = 8
HEADS_PER_CORE = 4
ROWS = 128 * HEADS_PER_CORE  # x rows per core
NM = 24                      # qkv column chunks of 128 (q:0-7, k:8-15, v:16-23)
KT = 8                       # contraction tiles over E
NG = 4                       # q groups of 512
NB = S // 128                # 16 blocks of 128 along s'


def build_program(with_qkv_bias: bool):
    nc = bacc.Bacc("TRN2", target_bir_lowering=False, debug=False,
                   num_devices=N_CORES)

    # x^T per head-pair: [hp, 128 (e%128), KT, 256 (rows)] bf16, host-built
    xt_in = nc.dram_tensor("xt", [2, 128, KT, 256], BF16, kind="ExternalInput")
    wqkv = nc.dram_tensor("wqkv", [NM, 128, KT, 128], BF16, kind="ExternalInput")
    wproj = nc.dram_tensor("wproj", [2, 128, E], BF16, kind="ExternalInput")
    identb_in = nc.dram_tensor("identb", [128, 64], BF16, kind="ExternalInput")
    triu_in = nc.dram_tensor("triu", [128, 128], BF16, kind="ExternalInput")
    if with_qkv_bias:
        bqkv = nc.dram_tensor("bqkv", [128, NM], F32, kind="ExternalInput")
    out = nc.dram_tensor("out", [S, E], F32, kind="ExternalOutput")

    with tile.TileContext(nc) as tc:
        with (
            tc.tile_pool(name="const", bufs=1) as constp,
            tc.tile_pool(name="persist", bufs=1) as persist,
        ):
            identb = constp.tile([128, 64], BF16)
            nc.sync.dma_start(identb[:], identb_in.ap())
            triu = constp.tile([128, 128], BF16)
            nc.sync.dma_start(triu[:], triu_in.ap())
            if with_qkv_bias:
                bias_sb = constp.tile([128, NM], F32)
                nc.sync.dma_start(bias_sb[:], bqkv.ap())

            wp_sb = [persist.tile([128, E], BF16, tag=f"wp{i}", name=f"wp{i}")
                     for i in range(2)]
            for i in range(2):
                nc.sync.dma_start(wp_sb[i][:], wproj.ap()[i])

            # x^T halves: [128, KT, 256] per head-pair
            xTh = [persist.tile([128, KT, 256], BF16, tag=f"xT{i}",
                                name=f"xT{i}") for i in range(2)]
            for i in range(2):
                nc.sync.dma_start(xTh[i][:], xt_in.ap()[i])

            # W_qkv SBUF-resident, streamed once (24 chunk DMAs)
            wq_sb = persist.tile([128, NM, KT, 128], BF16, tag="wq")
            for m in range(NM):
                nc.sync.dma_start(wq_sb[:, m, :, :], wqkv.ap()[m])

            # Q/K transposed per head-pair: [128 (2 heads x 64 d), 2048 (s')]
            qt = [persist.tile([128, S], F32R, tag=f"qt{i}", name=f"qt{i}")
                  for i in range(2)]
            kt_ = [persist.tile([128, S], F32R, tag=f"kt{i}", name=f"ktt{i}")
                   for i in range(2)]
            # V^T per head [64 (d), 2048 (s')], later transposed into vnat
            vt = [persist.tile([64, S], BF16, tag=f"vt{i}", name=f"vt{i}")
                  for i in range(4)]
            # V natural per head: 16 blocks of [128, 65] (col 64 = ones)
            vnat = [persist.tile([128, NB * 65], BF16, tag=f"vn{i}",
                                 name=f"vn{i}") for i in range(4)]
            # res^T per head-pair (normalized), bf16
            res = [persist.tile([128, S], BF16, tag=f"res{i}", name=f"res{i}")
                   for i in range(2)]

            with (
                tc.tile_pool(name="wk", bufs=2, space="PSUM") as wk,
                tc.tile_pool(name="avps", bufs=2, space="PSUM") as avps,
                tc.tile_pool(name="scps", bufs=2, space="PSUM") as scps,
                tc.tile_pool(name="expp", bufs=8) as expp,
                tc.tile_pool(name="nrm", bufs=4) as nrm,
                tc.tile_pool(name="osb", bufs=3) as osb,
            ):
                def add_bias(dst_ap, src_ap, m, partn=64):
                    base = src_ap.base_partition()
                    nc.vector.tensor_scalar_add(
                        dst_ap, src_ap, bias_sb[base:base + partn, m:m + 1])

                def qkv_copy(dst_ap, src_ap, on_act):
                    if on_act:
                        nc.scalar.copy(dst_ap, src_ap)
                    else:
                        nc.vector.tensor_copy(dst_ap, src_ap)

                def qkv_phase(hp):
                    # m-chunk pairs; ps[128, 512] = 2 chunks x 256 rows
                    for mp in range(NM // 2):
                        m0 = 2 * mp
                        ps = wk.tile([128, 512], F32, tag="wk",
                                     name=f"ps{hp}_{mp}")
                        for c in range(2):
                            for j in range(KT):
                                nc.tensor.matmul(
                                    ps[:, 256 * c:256 * c + 256],
                                    wq_sb[:, m0 + c, j, :],
                                    xTh[hp][:, j, :],
                                    start=(j == 0), stop=(j == KT - 1))
                        ps3 = ps[:].rearrange("p (ch r) -> p ch r", ch=2)
                        if with_qkv_bias:
                            # slow path: per-chunk bias adds (unmerged)
                            for c in range(2):
                                m = m0 + c
                                if m < 16:
                                    dstt = (qt if m < 8 else kt_)[hp]
                                    mm = m if m < 8 else m - 8
                                    dv = dstt[:].rearrange(
                                        "p (r cc) -> p r cc", cc=16)
                                    for pos in range(2):
                                        rb = ps[:, 256 * c + 128 * pos:
                                                256 * c + 128 * pos + 128]
                                        add_bias(dv[64 * pos:64 * pos + 64,
                                                    :, 2 * mm], rb[0:64, :], m)
                                        add_bias(dv[64 * pos:64 * pos + 64,
                                                    :, 2 * mm + 1],
                                                 rb[64:128, :], m)
                                else:
                                    mm = m - 16
                                    for pos in range(2):
                                        head = 2 * hp + pos
                                        dv = vt[head][:].rearrange(
                                            "p (r cc) -> p r cc", cc=16)
                                        rb = ps[:, 256 * c + 128 * pos:
                                                256 * c + 128 * pos + 128]
                                        add_bias(dv[:, :, 2 * mm],
                                                 rb[0:64, :], m)
                                        add_bias(dv[:, :, 2 * mm + 1],
                                                 rb[64:128, :], m)
                            continue
                        # fast path: merged pair copies
                        # ACT only takes copies in phase 0 (idle there);
                        # during phase 1 the ACT is saturated with exp.
                        if m0 < 16:
                            dstt = (qt if m0 < 8 else kt_)[hp]
                            mm0 = m0 if m0 < 8 else m0 - 8
                            dvt = dstt[:].rearrange(
                                "p (r cp two) -> p cp two r", two=2, cp=8)
                            for pos in range(2):
                                for sub in range(2):
                                    src = ps3[64 * sub:64 * sub + 64, :,
                                              128 * pos:128 * pos + 128]
                                    dst = dvt[64 * pos:64 * pos + 64,
                                              mm0:mm0 + 2, sub, :]
                                    qkv_copy(dst, src,
                                             on_act=(hp == 0 and pos == sub))
                        else:
                            mm0 = m0 - 16
                            for pos in range(2):
                                head = 2 * hp + pos
                                dvt = vt[head][:].rearrange(
                                    "p (r cp two) -> p cp two r", two=2, cp=8)
                                for sub in range(2):
                                    src = ps3[64 * sub:64 * sub + 64, :,
                                              128 * pos:128 * pos + 128]
                                    dst = dvt[:, mm0:mm0 + 2, sub, :]
                                    qkv_copy(dst, src,
                                             on_act=(hp == 0 and sub == 0))

                def vtrans_phase(hp):
                    # V blocks: transpose [64, 128] slices to natural order,
                    # two blocks per psum tile, one merged copy out.
                    for pos in range(2):
                        head = 2 * hp + pos
                        for jj in range(0, NB, 2):
                            vp = wk.tile([128, 512], BF16, tag="wk",
                                         name=f"vp{head}_{jj}")
                            nc.tensor.transpose(
                                vp[:, 0:64],
                                vt[head][:, 128 * jj:128 * jj + 128],
                                identb[0:64, :])
                            nc.tensor.transpose(
                                vp[:, 64:128],
                                vt[head][:, 128 * (jj + 1):128 * (jj + 1) + 128],
                                identb[0:64, :])
                            src = vp[:, 0:128].rearrange(
                                "p (b d) -> p b d", b=2)
                            dst = vnat[head][:].rearrange(
                                "p (b c) -> p b c", c=65)[:, jj:jj + 2, 0:64]
                            if jj % 4 == 0:
                                nc.vector.tensor_copy(dst, src)
                            else:
                                nc.scalar.copy(dst, src)
                        nc.vector.memset(
                            vnat[head][:].rearrange(
                                "p (jj c) -> p jj c", c=65)[:, :, 64], 1.0)

                def attn_g(hp, g):
                    q0 = 512 * g
                    av = [avps.tile([65, 512], F32, tag="av",
                                    name=f"av{hp}_{g}_{i}") for i in range(2)]
                    nkb = 4 * g + 4
                    for kb in range(nkb):
                        ingroup = kb >= 4 * g
                        coff = 128 * (kb - 4 * g) if ingroup else 0
                        sc = scps.tile([128, 1024], F32, tag="sc")
                        ex = expp.tile([128, 1024], BF16, tag="ex")
                        for pos in range(2):
                            so = 512 * pos
                            nc.tensor.matmul(
                                sc[:, so + coff:so + 512],
                                kt_[hp][64 * pos:64 * pos + 64,
                                        128 * kb:128 * kb + 128],
                                qt[hp][64 * pos:64 * pos + 64,
                                       q0 + coff:q0 + 512],
                                start=True, stop=True,
                                tile_position=(64 * pos, 0))
                        if not ingroup:
                            nc.scalar.activation(
                                ex[:], sc[:],
                                mybir.ActivationFunctionType.Exp,
                                scale=0.125)
                        else:
                            sc3 = sc[:].rearrange("p (s q) -> p s q", s=2)
                            ex3 = ex[:].rearrange("p (s q) -> p s q", s=2)
                            nc.scalar.activation(
                                ex3[:, :, coff:512],
                                sc3[:, :, coff:512],
                                mybir.ActivationFunctionType.Exp,
                                scale=0.125)
                            for pos in range(2):
                                so = 512 * pos
                                nc.gpsimd.tensor_mul(
                                    ex[:, so + coff:so + coff + 128],
                                    ex[:, so + coff:so + coff + 128],
                                    triu[:])
                        for pos in range(2):
                            so = 512 * pos
                            head = 2 * hp + pos
                            nc.tensor.matmul(
                                av[pos][:, coff:512],
                                vnat[head][:, 65 * kb:65 * kb + 65],
                                ex[:, so + coff:so + 512],
                                start=(kb == 0), stop=(kb == nkb - 1))
                    for pos in range(2):
                        den = nrm.tile([1, 512], F32, tag="den")
                        nc.vector.tensor_copy(den[:], av[pos][64:65, :])
                        rec = nrm.tile([1, 512], F32, tag="rec")
                        nc.vector.reciprocal_approx_fast(out=rec[:], in_=den[:])
                        bc = nrm.tile([64, 512], F32, tag="bc")
                        nc.gpsimd.partition_broadcast(bc[:], rec[:])
                        nc.vector.tensor_mul(
                            res[hp][64 * pos:64 * pos + 64, q0:q0 + 512],
                            av[pos][0:64, :], bc[:])

                def proj_group(gg):
                    for blk in range(4 * gg, 4 * gg + 4):
                        o = osb.tile([128, E], F32, name=f"o{blk}", tag="o")
                        for f in range(2):
                            pp = wk.tile([128, 512], F32, tag="wk",
                                         name=f"pp{blk}_{f}")
                            for hp in range(2):
                                nc.tensor.matmul(
                                    pp[:], res[hp][:, 128 * blk:128 * blk + 128],
                                    wp_sb[hp][:, 512 * f:512 * f + 512],
                                    start=(hp == 0), stop=(hp == 1))
                            if f == 0:
                                nc.vector.tensor_copy(o[:, 0:512], pp[:])
                            else:
                                nc.scalar.copy(o[:, 512:1024], pp[:])
                        nc.sync.dma_start(
                            out.ap()[128 * blk:128 * blk + 128, :], o[:])

                qkv_phase(0)
                vtrans_phase(0)
                for g in range(NG):
                    attn_g(0, g)
                qkv_phase(1)
                vtrans_phase(1)
                for g in range(NG):
                    attn_g(1, g)
                    proj_group(g)

    nc.compile()
    return nc


_CACHE = {}


def _get_program(with_qkv_bias: bool):
    if with_qkv_bias not in _CACHE:
        _CACHE[with_qkv_bias] = build_program(with_qkv_bias)
    return _CACHE[with_qkv_bias]


def make_in_maps(x, W_qkv, b_qkv, W_proj):
    """Build the 8 per-core input maps (host-side data marshaling only)."""
    x = np.ascontiguousarray(np.asarray(x, dtype=np.float32))
    W_qkv = np.asarray(W_qkv, dtype=np.float32)
    b_qkv = np.asarray(b_qkv, dtype=np.float32)
    W_proj = np.asarray(W_proj, dtype=np.float32)

    wq_t = np.ascontiguousarray(
        W_qkv.astype(ml_dtypes.bfloat16).reshape(KT, 128, NM, 128)
        .transpose(2, 1, 0, 3))
    wp_b = W_proj.astype(ml_dtypes.bfloat16)
    identb = np.vstack([np.eye(64), np.eye(64)]).astype(ml_dtypes.bfloat16)
    # causal mask for diagonal blocks: visible k<=q
    triu = np.triu(np.ones((128, 128), np.float32)).astype(ml_dtypes.bfloat16)
    with_bias = bool(np.any(b_qkv))
    bias_t = np.ascontiguousarray(b_qkv.reshape(NM, 128).T) if with_bias else None

    x_bf = x.astype(ml_dtypes.bfloat16)
    in_maps = []
    for c in range(N_CORES):
        b, qi = c // 4, c % 4
        rows = x_bf[b, ROWS * qi:ROWS * qi + ROWS, :]       # [512, 1024]
        # xt[hp][p, j, r] = rows[256*hp + r, 128*j + p]
        xtT = np.ascontiguousarray(
            rows.T.reshape(KT, 128, ROWS).transpose(1, 0, 2))  # [128, KT, 512]
        xt = np.ascontiguousarray(
            np.stack([xtT[:, :, 0:256], xtT[:, :, 256:512]]))  # [2,128,KT,256]
        m = {
            "xt": xt,
            "wqkv": wq_t,
            "wproj": np.ascontiguousarray(
                wp_b[256 * qi:256 * qi + 256, :].reshape(2, 128, E)),
            "identb": identb,
            "triu": triu,
        }
        if with_bias:
            m["bqkv"] = bias_t
        in_maps.append(m)
    return in_maps, with_bias


def kernel(x, W_qkv, b_qkv, W_proj, b_proj, _run_kwargs=None):
    in_maps, with_bias = make_in_maps(x, W_qkv, b_qkv, W_proj)
    nc = _get_program(with_bias)
    res = bass_utils.run_bass_kernel_spmd(
        nc, in_maps, core_ids=list(range(N_CORES)), **(_run_kwargs or {}))
    out = np.zeros((B, S, E), np.float32)
    for c in range(N_CORES):
        out[c // 4] += res.results[c]["out"]
    out += np.asarray(b_proj, dtype=np.float32)[None, None, :]
    if _run_kwargs:
        kernel.last_results = res
    return out


# revision 14
# speedup vs baseline: 1.3738x; 1.3738x over previous
"""Trainium2 Bass kernel for causal self-attention (nn_Casualselfatt).

Reference computes (B=2, S=2048, E=1024, H=16, D=64, fp32):
    qkv = x @ W_qkv + b_qkv ; q,k,v = split(qkv)
    q = q.reshape(B, H, S, D)   # NOTE: raw reshape, no transpose.
    ...causal softmax attention per (b,h)...
    out = res @ W_proj + b_proj

The raw reshape means head h of batch b attends over the [S, D] reshape of
rows [128h, 128h+128) of q/k/v[b].  Sharding: 32 (b,h) pairs -> 4 heads of
one batch per core (core c: b=c//4, heads 4*(c%4)..+4).  Each core computes
a partial projection output; the host sums 4 partials per batch.

On-chip: scores are built transposed ([k-part, q-free]) so the softmax
denominator rides an appended ones-column through the AV matmul.  QKV runs
in bf16 (fp32 accumulate); scores run in float32r; the post-softmax path is
bf16.  x arrives pre-transposed from the host (bf16), W_qkv is SBUF-resident
(streamed once), and QKV psum->SBUF copies are pair-merged and split across
DVE/ACT.  PSUM: sc pool 4 banks + shared 1-bank work pool (qkv ps / av /
vtrans vp / proj pp) x4 bufs so attention groups double-buffer across g.
Softmax reciprocal uses the fast approx DVE op (~5x cheaper).
"""

import numpy as np
import ml_dtypes

import concourse.bass as bass
import concourse.tile as tile
from concourse import bacc, mybir
import concourse.bass_utils as bass_utils

F32 = mybir.dt.float32
F32R = mybir.dt.float32r
BF16 = mybir.dt.bfloat16

B, S, E = 2, 2048, 1024
H, D = 16, 64
N_CORES = 8
HEADS_PER_CORE = 4
ROWS = 128 * HEADS_PER_CORE  # x rows per core
NM = 24                      # qkv column chunks of 128 (q:0-7, k:8-15, v:16-23)
KT = 8                       # contraction tiles over E
NG = 4                       # q groups of 512
NB = S // 128                # 16 blocks of 128 along s'


def build_program(with_qkv_bias: bool):
    nc = bacc.Bacc("TRN2", target_bir_lowering=False, debug=False,
                   num_devices=N_CORES)

    # x^T per head-pair: [hp, 128 (e%128), KT, 256 (rows)] bf16, host-built
    xt_in = nc.dram_tensor("xt", [2, 128, KT, 256], BF16, kind="ExternalInput")
    wqkv = nc.dram_tensor("wqkv", [NM, 128, KT, 128], BF16, kind="ExternalInput")
    wproj = nc.dram_tensor("wproj", [2, 128, E], BF16, kind="ExternalInput")
    identb_in = nc.dram_tensor("identb", [128, 64], BF16, kind="ExternalInput")
    triu_in = nc.dram_tensor("triu", [128, 128], BF16, kind="ExternalInput")
    if with_qkv_bias:
        bqkv = nc.dram_tensor("bqkv", [128, NM], F32, kind="ExternalInput")
    out = nc.dram_tensor("out", [S, E], F32, kind="ExternalOutput")

    with tile.TileContext(nc) as tc:
        with (
            tc.tile_pool(name="const", bufs=1) as constp,
            tc.tile_pool(name="persist", bufs=1) as persist,
        ):
            # QKV-critical data first: x^T halves then W_qkv chunks.
            xTh = [persist.tile([128, KT, 256], BF16, tag=f"xT{i}",
                                name=f"xT{i}") for i in range(2)]
            nc.sync.dma_start(xTh[0][:], xt_in.ap()[0])
            # W_qkv SBUF-resident, streamed once (24 chunk DMAs)
            wq_sb = persist.tile([128, NM, KT, 128], BF16, tag="wq")
            for m in range(4):
                nc.sync.dma_start(wq_sb[:, m, :, :], wqkv.ap()[m])
            nc.sync.dma_start(xTh[1][:], xt_in.ap()[1])
            for m in range(4, NM):
                nc.sync.dma_start(wq_sb[:, m, :, :], wqkv.ap()[m])

            identb = constp.tile([128, 64], BF16)
            nc.sync.dma_start(identb[:], identb_in.ap())
            triu = constp.tile([128, 128], BF16)
            nc.sync.dma_start(triu[:], triu_in.ap())
            if with_qkv_bias:
                bias_sb = constp.tile([128, NM], F32)
                nc.sync.dma_start(bias_sb[:], bqkv.ap())

            wp_sb = [persist.tile([128, E], BF16, tag=f"wp{i}", name=f"wp{i}")
                     for i in range(2)]
            for i in range(2):
                nc.sync.dma_start(wp_sb[i][:], wproj.ap()[i])

            # Q/K transposed per head-pair: [128 (2 heads x 64 d), 2048 (s')]
            qt = [persist.tile([128, S], F32R, tag=f"qt{i}", name=f"qt{i}")
                  for i in range(2)]
            kt_ = [persist.tile([128, S], F32R, tag=f"kt{i}", name=f"ktt{i}")
                   for i in range(2)]
            # V^T per head [64 (d), 2048 (s')], later transposed into vnat
            vt = [persist.tile([64, S], BF16, tag=f"vt{i}", name=f"vt{i}")
                  for i in range(4)]
            # V natural per head: 16 blocks of [128, 65] (col 64 = ones)
            vnat = [persist.tile([128, NB * 65], BF16, tag=f"vn{i}",
                                 name=f"vn{i}") for i in range(4)]
            # res^T per head-pair (normalized), bf16
            res = [persist.tile([128, S], BF16, tag=f"res{i}", name=f"res{i}")
                   for i in range(2)]

            with (
                tc.tile_pool(name="wk", bufs=2, space="PSUM") as wk,
                tc.tile_pool(name="avps", bufs=2, space="PSUM") as avps,
                tc.tile_pool(name="scps", bufs=2, space="PSUM") as scps,
                tc.tile_pool(name="expp", bufs=8) as expp,
                tc.tile_pool(name="nrm", bufs=4) as nrm,
                tc.tile_pool(name="osb", bufs=3) as osb,
            ):
                def add_bias(dst_ap, src_ap, m, partn=64):
                    base = src_ap.base_partition()
                    nc.vector.tensor_scalar_add(
                        dst_ap, src_ap, bias_sb[base:base + partn, m:m + 1])

                def qkv_copy(dst_ap, src_ap, on_act):
                    if on_act:
                        nc.scalar.copy(dst_ap, src_ap)
                    else:
                        nc.vector.tensor_copy(dst_ap, src_ap)

                def qkv_phase(hp):
                    # m-chunk pairs; ps[128, 512] = 2 chunks x 256 rows
                    for mp in range(NM // 2):
                        m0 = 2 * mp
                        ps = wk.tile([128, 512], F32, tag="wk",
                                     name=f"ps{hp}_{mp}")
                        for c in range(2):
                            for j in range(KT):
                                nc.tensor.matmul(
                                    ps[:, 256 * c:256 * c + 256],
                                    wq_sb[:, m0 + c, j, :],
                                    xTh[hp][:, j, :],
                                    start=(j == 0), stop=(j == KT - 1))
                        ps3 = ps[:].rearrange("p (ch r) -> p ch r", ch=2)
                        if with_qkv_bias:
                            # slow path: per-chunk bias adds (unmerged)
                            for c in range(2):
                                m = m0 + c
                                if m < 16:
                                    dstt = (qt if m < 8 else kt_)[hp]
                                    mm = m if m < 8 else m - 8
                                    dv = dstt[:].rearrange(
                                        "p (r cc) -> p r cc", cc=16)
                                    for pos in range(2):
                                        rb = ps[:, 256 * c + 128 * pos:
                                                256 * c + 128 * pos + 128]
                                        add_bias(dv[64 * pos:64 * pos + 64,
                                                    :, 2 * mm], rb[0:64, :], m)
                                        add_bias(dv[64 * pos:64 * pos + 64,
                                                    :, 2 * mm + 1],
                                                 rb[64:128, :], m)
                                else:
                                    mm = m - 16
                                    for pos in range(2):
                                        head = 2 * hp + pos
                                        dv = vt[head][:].rearrange(
                                            "p (r cc) -> p r cc", cc=16)
                                        rb = ps[:, 256 * c + 128 * pos:
                                                256 * c + 128 * pos + 128]
                                        add_bias(dv[:, :, 2 * mm],
                                                 rb[0:64, :], m)
                                        add_bias(dv[:, :, 2 * mm + 1],
                                                 rb[64:128, :], m)
                            continue
                        # fast path: merged pair copies
                        # ACT only takes copies in phase 0 (idle there);
                        # during phase 1 the ACT is saturated with exp.
                        if m0 < 16:
                            dstt = (qt if m0 < 8 else kt_)[hp]
                            mm0 = m0 if m0 < 8 else m0 - 8
                            dvt = dstt[:].rearrange(
                                "p (r cp two) -> p cp two r", two=2, cp=8)
                            for pos in range(2):
                                for sub in range(2):
                                    src = ps3[64 * sub:64 * sub + 64, :,
                                              128 * pos:128 * pos + 128]
                                    dst = dvt[64 * pos:64 * pos + 64,
                                              mm0:mm0 + 2, sub, :]
                                    qkv_copy(dst, src,
                                             on_act=(hp == 0 and pos == sub))
                        else:
                            mm0 = m0 - 16
                            for pos in range(2):
                                head = 2 * hp + pos
                                dvt = vt[head][:].rearrange(
                                    "p (r cp two) -> p cp two r", two=2, cp=8)
                                for sub in range(2):
                                    src = ps3[64 * sub:64 * sub + 64, :,
                                              128 * pos:128 * pos + 128]
                                    dst = dvt[:, mm0:mm0 + 2, sub, :]
                                    qkv_copy(dst, src,
                                             on_act=(hp == 0 and sub == 0))

                def vtrans_phase(hp):
                    # V blocks: transpose [64, 128] slices to natural order,
                    # two blocks per psum tile, one merged copy out.
                    for pos in range(2):
                        head = 2 * hp + pos
                        for jj in range(0, NB, 2):
                            vp = wk.tile([128, 512], BF16, tag="wk",
                                         name=f"vp{head}_{jj}")
                            nc.tensor.transpose(
                                vp[:, 0:64],
                                vt[head][:, 128 * jj:128 * jj + 128],
                                identb[0:64, :])
                            nc.tensor.transpose(
                                vp[:, 64:128],
                                vt[head][:, 128 * (jj + 1):128 * (jj + 1) + 128],
                                identb[0:64, :])
                            src = vp[:, 0:128].rearrange(
                                "p (b d) -> p b d", b=2)
                            dst = vnat[head][:].rearrange(
                                "p (b c) -> p b c", c=65)[:, jj:jj + 2, 0:64]
                            if jj % 4 == 0:
                                nc.vector.tensor_copy(dst, src)
                            else:
                                nc.scalar.copy(dst, src)
                        nc.vector.memset(
                            vnat[head][:].rearrange(
                                "p (jj c) -> p jj c", c=65)[:, :, 64], 1.0)

                def attn_g(hp, g):
                    q0 = 512 * g
                    av = [avps.tile([65, 512], F32, tag="av",
                                    name=f"av{hp}_{g}_{i}") for i in range(2)]
                    nkb = 4 * g + 4
                    for kb in range(nkb):
                        ingroup = kb >= 4 * g
                        coff = 128 * (kb - 4 * g) if ingroup else 0
                        sc = scps.tile([128, 1024], F32, tag="sc")
                        ex = expp.tile([128, 1024], BF16, tag="ex")
                        for pos in range(2):
                            so = 512 * pos
                            nc.tensor.matmul(
                                sc[:, so + coff:so + 512],
                                kt_[hp][64 * pos:64 * pos + 64,
                                        128 * kb:128 * kb + 128],
                                qt[hp][64 * pos:64 * pos + 64,
                                       q0 + coff:q0 + 512],
                                start=True, stop=True,
                                tile_position=(64 * pos, 0))
                        if not ingroup:
                            nc.scalar.activation(
                                ex[:], sc[:],
                                mybir.ActivationFunctionType.Exp,
                                scale=0.125)
                        else:
                            sc3 = sc[:].rearrange("p (s q) -> p s q", s=2)
                            ex3 = ex[:].rearrange("p (s q) -> p s q", s=2)
                            nc.scalar.activation(
                                ex3[:, :, coff:512],
                                sc3[:, :, coff:512],
                                mybir.ActivationFunctionType.Exp,
                                scale=0.125)
                            for pos in range(2):
                                so = 512 * pos
                                nc.vector.tensor_mul(
                                    ex[:, so + coff:so + coff + 128],
                                    ex[:, so + coff:so + coff + 128],
                                    triu[:])
                        for pos in range(2):
                            so = 512 * pos
                            head = 2 * hp + pos
                            nc.tensor.matmul(
                                av[pos][:, coff:512],
                                vnat[head][:, 65 * kb:65 * kb + 65],
                                ex[:, so + coff:so + 512],
                                start=(kb == 0), stop=(kb == nkb - 1))
                    for pos in range(2):
                        # evacuate av -> SBUF promptly so the psum slot frees
                        den = nrm.tile([1, 512], F32, tag="den")
                        nc.vector.tensor_copy(den[:], av[pos][64:65, :])
                        avs = nrm.tile([64, 512], F32, tag="avs")
                        nc.vector.tensor_copy(avs[:], av[pos][0:64, :])
                        rec = nrm.tile([1, 512], F32, tag="rec")
                        nc.vector.reciprocal_approx_fast(
                            out=rec[:], in_=den[:])
                        bc = nrm.tile([64, 512], F32, tag="bc")
                        nc.gpsimd.partition_broadcast(bc[:], rec[:])
                        nc.vector.tensor_mul(
                            res[hp][64 * pos:64 * pos + 64, q0:q0 + 512],
                            avs[:], bc[:])

                def proj_group(gg):
                    for blk in range(4 * gg, 4 * gg + 4):
                        o = osb.tile([128, E], F32, name=f"o{blk}", tag="o")
                        for f in range(2):
                            pp = wk.tile([128, 512], F32, tag="wk",
                                         name=f"pp{blk}_{f}")
                            for hp in range(2):
                                nc.tensor.matmul(
                                    pp[:], res[hp][:, 128 * blk:128 * blk + 128],
                                    wp_sb[hp][:, 512 * f:512 * f + 512],
                                    start=(hp == 0), stop=(hp == 1))
                            if f == 0:
                                nc.vector.tensor_copy(o[:, 0:512], pp[:])
                            else:
                                nc.scalar.copy(o[:, 512:1024], pp[:])
                        nc.sync.dma_start(
                            out.ap()[128 * blk:128 * blk + 128, :], o[:])

                qkv_phase(0)
                vtrans_phase(0)
                for g in range(NG):
                    attn_g(0, g)
                qkv_phase(1)
                vtrans_phase(1)
                for g in range(NG):
                    attn_g(1, g)
                    proj_group(g)

    nc.compile()
    return nc


_CACHE = {}


def _get_program(with_qkv_bias: bool):
    if with_qkv_bias not in _CACHE:
        _CACHE[with_qkv_bias] = build_program(with_qkv_bias)
    return _CACHE[with_qkv_bias]


def make_in_maps(x, W_qkv, b_qkv, W_proj):
    """Build the 8 per-core input maps (host-side data marshaling only)."""
    x = np.ascontiguousarray(np.asarray(x, dtype=np.float32))
    W_qkv = np.asarray(W_qkv, dtype=np.float32)
    b_qkv = np.asarray(b_qkv, dtype=np.float32)
    W_proj = np.asarray(W_proj, dtype=np.float32)

    wq_t = np.ascontiguousarray(
        W_qkv.astype(ml_dtypes.bfloat16).reshape(KT, 128, NM, 128)
        .transpose(2, 1, 0, 3))
    wp_b = W_proj.astype(ml_dtypes.bfloat16)
    identb = np.vstack([np.eye(64), np.eye(64)]).astype(ml_dtypes.bfloat16)
    # causal mask for diagonal blocks: visible k<=q
    triu = np.triu(np.ones((128, 128), np.float32)).astype(ml_dtypes.bfloat16)
    with_bias = bool(np.any(b_qkv))
    bias_t = np.ascontiguousarray(b_qkv.reshape(NM, 128).T) if with_bias else None

    x_bf = x.astype(ml_dtypes.bfloat16)
    in_maps = []
    for c in range(N_CORES):
        b, qi = c // 4, c % 4
        rows = x_bf[b, ROWS * qi:ROWS * qi + ROWS, :]       # [512, 1024]
        # xt[hp][p, j, r] = rows[256*hp + r, 128*j + p]
        xtT = np.ascontiguousarray(
            rows.T.reshape(KT, 128, ROWS).transpose(1, 0, 2))  # [128, KT, 512]
        xt = np.ascontiguousarray(
            np.stack([xtT[:, :, 0:256], xtT[:, :, 256:512]]))  # [2,128,KT,256]
        m = {
            "xt": xt,
            "wqkv": wq_t,
            "wproj": np.ascontiguousarray(
                wp_b[256 * qi:256 * qi + 256, :].reshape(2, 128, E)),
            "identb": identb,
            "triu": triu,
        }
        if with_bias:
            m["bqkv"] = bias_t
        in_maps.append(m)
    return in_maps, with_bias


def kernel(x, W_qkv, b_qkv, W_proj, b_proj, _run_kwargs=None):
    in_maps, with_bias = make_in_maps(x, W_qkv, b_qkv, W_proj)
    nc = _get_program(with_bias)
    res = bass_utils.run_bass_kernel_spmd(
        nc, in_maps, core_ids=list(range(N_CORES)), **(_run_kwargs or {}))
    out = np.zeros((B, S, E), np.float32)
    for c in range(N_CORES):
        out[c // 4] += res.results[c]["out"]
    out += np.asarray(b_proj, dtype=np.float32)[None, None, :]
    if _run_kwargs:
        kernel.last_results = res
    return out


# revision 20
# speedup vs baseline: 1.4315x; 1.0420x over previous
"""Trainium2 Bass kernel for causal self-attention (nn_Casualselfatt).

Reference computes (B=2, S=2048, E=1024, H=16, D=64, fp32):
    qkv = x @ W_qkv + b_qkv ; q,k,v = split(qkv)
    q = q.reshape(B, H, S, D)   # NOTE: raw reshape, no transpose.
    ...causal softmax attention per (b,h)...
    out = res @ W_proj + b_proj

The raw reshape means head h of batch b attends over the [S, D] reshape of
rows [128h, 128h+128) of q/k/v[b].  Sharding: 32 (b,h) pairs -> 4 heads of
one batch per core (core c: b=c//4, heads 4*(c%4)..+4).  Each core computes
a partial projection output; the host sums 4 partials per batch.

On-chip: scores are built transposed ([k-part, q-free]) so the softmax
denominator rides an appended ones-column through the AV matmul.  QKV runs
in bf16 (fp32 accumulate); scores run in float32r; the post-softmax path is
bf16.  x arrives pre-transposed from the host (bf16), W_qkv is SBUF-resident
(streamed once), and QKV psum->SBUF copies are pair-merged and split across
DVE/ACT.  PSUM: sc pool 4 banks + shared 1-bank work pool (qkv ps / av /
vtrans vp / proj pp) x4 bufs so attention groups double-buffer across g.
Softmax reciprocal uses the fast approx DVE op (~5x cheaper).
"""

import numpy as np
import ml_dtypes

import concourse.bass as bass
import concourse.tile as tile
from concourse import bacc, mybir
import concourse.bass_utils as bass_utils

F32 = mybir.dt.float32
F32R = mybir.dt.float32r
BF16 = mybir.dt.bfloat16

B, S, E = 2, 2048, 1024
H, D = 16, 64
N_CORES = 8
HEADS_PER_CORE = 4
ROWS = 128 * HEADS_PER_CORE  # x rows per core
NM = 24                      # qkv column chunks of 128 (q:0-7, k:8-15, v:16-23)
KT = 8                       # contraction tiles over E
NG = 4                       # q groups of 512
NB = S // 128                # 16 blocks of 128 along s'


def build_program(with_qkv_bias: bool):
    nc = bacc.Bacc("TRN2", target_bir_lowering=False, debug=False,
                   num_devices=N_CORES)

    # x^T per head-pair: [hp, 128 (e%128), KT, 256 (rows)] bf16, host-built
    xt_in = nc.dram_tensor("xt", [2, 128, KT, 256], BF16, kind="ExternalInput")
    wqkv = nc.dram_tensor("wqkv", [NM, 128, KT, 128], BF16, kind="ExternalInput")
    wproj = nc.dram_tensor("wproj", [2, 128, E], BF16, kind="ExternalInput")
    identb_in = nc.dram_tensor("identb", [128, 64], BF16, kind="ExternalInput")
    triu_in = nc.dram_tensor("triu", [128, 128], BF16, kind="ExternalInput")
    if with_qkv_bias:
        bqkv = nc.dram_tensor("bqkv", [128, NM], F32, kind="ExternalInput")
    out = nc.dram_tensor("out", [S, E], F32, kind="ExternalOutput")

    with tile.TileContext(nc) as tc:
        with (
            tc.tile_pool(name="const", bufs=1) as constp,
            tc.tile_pool(name="persist", bufs=1) as persist,
        ):
            # QKV-critical data first: x^T halves then W_qkv chunks.
            xTh = [persist.tile([128, KT, 256], BF16, tag=f"xT{i}",
                                name=f"xT{i}") for i in range(2)]
            nc.sync.dma_start(xTh[0][:], xt_in.ap()[0])
            # W_qkv SBUF-resident, streamed once (24 chunk DMAs)
            wq_sb = persist.tile([128, NM, KT, 128], BF16, tag="wq")
            for m in range(4):
                nc.sync.dma_start(wq_sb[:, m, :, :], wqkv.ap()[m])
            nc.sync.dma_start(xTh[1][:], xt_in.ap()[1])
            for m in range(4, NM):
                nc.sync.dma_start(wq_sb[:, m, :, :], wqkv.ap()[m])

            identb = constp.tile([128, 64], BF16)
            nc.sync.dma_start(identb[:], identb_in.ap())
            triu = constp.tile([128, 128], BF16)
            nc.sync.dma_start(triu[:], triu_in.ap())
            if with_qkv_bias:
                bias_sb = constp.tile([128, NM], F32)
                nc.sync.dma_start(bias_sb[:], bqkv.ap())

            wp_sb = [persist.tile([128, E], BF16, tag=f"wp{i}", name=f"wp{i}")
                     for i in range(2)]
            for i in range(2):
                nc.sync.dma_start(wp_sb[i][:], wproj.ap()[i])

            # Q/K transposed per head-pair: [128 (2 heads x 64 d), 2048 (s')]
            qt = [persist.tile([128, S], F32R, tag=f"qt{i}", name=f"qt{i}")
                  for i in range(2)]
            kt_ = [persist.tile([128, S], F32R, tag=f"kt{i}", name=f"ktt{i}")
                   for i in range(2)]
            # V^T per head [64 (d), 2048 (s')], later transposed into vnat
            vt = [persist.tile([64, S], BF16, tag=f"vt{i}", name=f"vt{i}")
                  for i in range(4)]
            # V natural per head: 16 blocks of [128, 65] (col 64 = ones)
            vnat = [persist.tile([128, NB * 65], BF16, tag=f"vn{i}",
                                 name=f"vn{i}") for i in range(4)]
            # res^T per head-pair (normalized), bf16
            res = [persist.tile([128, S], BF16, tag=f"res{i}", name=f"res{i}")
                   for i in range(2)]

            with (
                tc.tile_pool(name="wk", bufs=2, space="PSUM") as wk,
                tc.tile_pool(name="avps", bufs=2, space="PSUM") as avps,
                tc.tile_pool(name="scps", bufs=2, space="PSUM") as scps,
                tc.tile_pool(name="expp", bufs=8) as expp,
                tc.tile_pool(name="nrm", bufs=4) as nrm,
                tc.tile_pool(name="osb", bufs=3) as osb,
            ):
                def add_bias(dst_ap, src_ap, m, partn=64):
                    base = src_ap.base_partition()
                    nc.vector.tensor_scalar_add(
                        dst_ap, src_ap, bias_sb[base:base + partn, m:m + 1])

                def qkv_copy(dst_ap, src_ap, on_act):
                    if on_act:
                        nc.scalar.copy(dst_ap, src_ap)
                    else:
                        nc.vector.tensor_copy(dst_ap, src_ap)

                def qkv_phase(hp):
                    # m-chunk pairs; ps[128, 512] = 2 chunks x 256 rows.
                    # V pairs first so vtrans can overlap the q/k copy drain.
                    order = list(range(8, 12)) + list(range(8))
                    for mp in order:
                        m0 = 2 * mp
                        ps = wk.tile([128, 512], F32, tag="wk",
                                     name=f"ps{hp}_{mp}")
                        for c in range(2):
                            for j in range(KT):
                                nc.tensor.matmul(
                                    ps[:, 256 * c:256 * c + 256],
                                    wq_sb[:, m0 + c, j, :],
                                    xTh[hp][:, j, :],
                                    start=(j == 0), stop=(j == KT - 1))
                        ps4 = ps[:].rearrange("p (ch po jj rl) -> p ch po jj rl",
                                              ch=2, po=2, rl=8)
                        if with_qkv_bias:
                            # slow path: per-chunk bias adds (unmerged)
                            for c in range(2):
                                m = m0 + c
                                if m < 16:
                                    dstt = (qt if m < 8 else kt_)[hp]
                                    mm = m if m < 8 else m - 8
                                    dv = dstt[:].rearrange(
                                        "p (jj sb rl) -> p sb jj rl",
                                        sb=16, rl=8)
                                    for pos in range(2):
                                        rb = ps[:, 256 * c + 128 * pos:
                                                256 * c + 128 * pos + 128]
                                        rb3 = rb.rearrange(
                                            "p (jj rl) -> p jj rl", rl=8)
                                        add_bias(dv[64 * pos:64 * pos + 64,
                                                    2 * mm, :, :],
                                                 rb3[0:64, :, :], m)
                                        add_bias(dv[64 * pos:64 * pos + 64,
                                                    2 * mm + 1, :, :],
                                                 rb3[64:128, :, :], m)
                                else:
                                    mm = m - 16
                                    for pos in range(2):
                                        head = 2 * hp + pos
                                        dv = vt[head][:].rearrange(
                                            "p (jj sb rl) -> p sb jj rl",
                                            sb=16, rl=8)
                                        rb = ps[:, 256 * c + 128 * pos:
                                                256 * c + 128 * pos + 128]
                                        rb3 = rb.rearrange(
                                            "p (jj rl) -> p jj rl", rl=8)
                                        add_bias(dv[:, 2 * mm, :, :],
                                                 rb3[0:64, :, :], m)
                                        add_bias(dv[:, 2 * mm + 1, :, :],
                                                 rb3[64:128, :, :], m)
                            continue
                        # fast path: merged pair copies
                        # ACT only takes copies in phase 0 (idle there);
                        # during phase 1 the ACT is saturated with exp.
                        if m0 < 16:
                            dstt = (qt if m0 < 8 else kt_)[hp]
                            mm0 = m0 if m0 < 8 else m0 - 8
                            dvt = dstt[:].rearrange(
                                "p (jj cp two rl) -> p cp two jj rl",
                                cp=8, two=2, rl=8)
                            for pos in range(2):
                                for sub in range(2):
                                    src = ps4[64 * sub:64 * sub + 64, :,
                                              pos, :, :]
                                    dst = dvt[64 * pos:64 * pos + 64,
                                              mm0:mm0 + 2, sub, :, :]
                                    qkv_copy(dst, src,
                                             on_act=(hp == 0 and pos == sub))
                        else:
                            mm0 = m0 - 16
                            for pos in range(2):
                                head = 2 * hp + pos
                                dvt = vt[head][:].rearrange(
                                    "p (jj cp two rl) -> p cp two jj rl",
                                    cp=8, two=2, rl=8)
                                for sub in range(2):
                                    src = ps4[64 * sub:64 * sub + 64, :,
                                              pos, :, :]
                                    dst = dvt[:, mm0:mm0 + 2, sub, :, :]
                                    qkv_copy(dst, src,
                                             on_act=(hp == 0 and sub == 0))

                def vtrans_phase(hp):
                    # V blocks: transpose [64, 128] slices to natural order,
                    # two blocks per psum tile, one merged copy out.
                    for pos in range(2):
                        head = 2 * hp + pos
                        for jj in range(0, NB, 2):
                            vp = wk.tile([128, 512], BF16, tag="wk",
                                         name=f"vp{head}_{jj}")
                            nc.tensor.transpose(
                                vp[:, 0:64],
                                vt[head][:, 128 * jj:128 * jj + 128],
                                identb[0:64, :])
                            nc.tensor.transpose(
                                vp[:, 64:128],
                                vt[head][:, 128 * (jj + 1):128 * (jj + 1) + 128],
                                identb[0:64, :])
                            src = vp[:, 0:128].rearrange(
                                "p (b d) -> p b d", b=2)
                            dst = vnat[head][:].rearrange(
                                "p (b c) -> p b c", c=65)[:, jj:jj + 2, 0:64]
                            if jj % 4 == 0:
                                nc.vector.tensor_copy(dst, src)
                            else:
                                nc.scalar.copy(dst, src)
                        nc.vector.memset(
                            vnat[head][:].rearrange(
                                "p (jj c) -> p jj c", c=65)[:, :, 64], 1.0)

                def attn_g(hp, g):
                    q0 = 512 * g
                    av = [avps.tile([65, 512], F32, tag="av",
                                    name=f"av{hp}_{g}_{i}") for i in range(2)]
                    nkb = 4 * g + 4
                    for kb in range(nkb):
                        ingroup = kb >= 4 * g
                        coff = 128 * (kb - 4 * g) if ingroup else 0
                        sc = scps.tile([128, 1024], F32, tag="sc")
                        ex = expp.tile([128, 1024], BF16, tag="ex")
                        for pos in range(2):
                            so = 512 * pos
                            nc.tensor.matmul(
                                sc[:, so + coff:so + 512],
                                kt_[hp][64 * pos:64 * pos + 64,
                                        128 * kb:128 * kb + 128],
                                qt[hp][64 * pos:64 * pos + 64,
                                       q0 + coff:q0 + 512],
                                start=True, stop=True,
                                tile_position=(64 * pos, 0))
                        if not ingroup:
                            nc.scalar.activation(
                                ex[:], sc[:],
                                mybir.ActivationFunctionType.Exp,
                                scale=0.125)
                        else:
                            sc3 = sc[:].rearrange("p (s q) -> p s q", s=2)
                            ex3 = ex[:].rearrange("p (s q) -> p s q", s=2)
                            nc.scalar.activation(
                                ex3[:, :, coff:512],
                                sc3[:, :, coff:512],
                                mybir.ActivationFunctionType.Exp,
                                scale=0.125)
                            for pos in range(2):
                                so = 512 * pos
                                nc.vector.tensor_mul(
                                    ex[:, so + coff:so + coff + 128],
                                    ex[:, so + coff:so + coff + 128],
                                    triu[:])
                        for pos in range(2):
                            so = 512 * pos
                            head = 2 * hp + pos
                            nc.tensor.matmul(
                                av[pos][:, coff:512],
                                vnat[head][:, 65 * kb:65 * kb + 65],
                                ex[:, so + coff:so + 512],
                                start=(kb == 0), stop=(kb == nkb - 1))
                    for pos in range(2):
                        # evacuate av -> SBUF promptly so the psum slot frees
                        den = nrm.tile([1, 512], F32, tag="den")
                        nc.vector.tensor_copy(den[:], av[pos][64:65, :])
                        avs = nrm.tile([64, 512], F32, tag="avs")
                        nc.vector.tensor_copy(avs[:], av[pos][0:64, :])
                        rec = nrm.tile([1, 512], F32, tag="rec")
                        nc.vector.reciprocal_approx_fast(
                            out=rec[:], in_=den[:])
                        bc = nrm.tile([64, 512], F32, tag="bc")
                        nc.gpsimd.partition_broadcast(bc[:], rec[:])
                        nc.vector.tensor_mul(
                            res[hp][64 * pos:64 * pos + 64, q0:q0 + 512],
                            avs[:], bc[:])

                def proj_group(gg):
                    for blk in range(4 * gg, 4 * gg + 4):
                        o = osb.tile([128, E], F32, name=f"o{blk}", tag="o")
                        for f in range(2):
                            pp = wk.tile([128, 512], F32, tag="wk",
                                         name=f"pp{blk}_{f}")
                            for hp in range(2):
                                nc.tensor.matmul(
                                    pp[:], res[hp][:, 128 * blk:128 * blk + 128],
                                    wp_sb[hp][:, 512 * f:512 * f + 512],
                                    start=(hp == 0), stop=(hp == 1))
                            if f == 0:
                                nc.vector.tensor_copy(o[:, 0:512], pp[:])
                            else:
                                nc.scalar.copy(o[:, 512:1024], pp[:])
                        nc.sync.dma_start(
                            out.ap()[128 * blk:128 * blk + 128, :], o[:])

                qkv_phase(0)
                vtrans_phase(0)
                for g in range(NG):
                    attn_g(0, g)
                qkv_phase(1)
                vtrans_phase(1)
                for g in range(NG):
                    attn_g(1, g)
                    proj_group(g)

    nc.compile()
    return nc


_CACHE = {}


def _get_program(with_qkv_bias: bool):
    if with_qkv_bias not in _CACHE:
        _CACHE[with_qkv_bias] = build_program(with_qkv_bias)
    return _CACHE[with_qkv_bias]


def make_in_maps(x, W_qkv, b_qkv, W_proj):
    """Build the 8 per-core input maps (host-side data marshaling only)."""
    x = np.ascontiguousarray(np.asarray(x, dtype=np.float32))
    W_qkv = np.asarray(W_qkv, dtype=np.float32)
    b_qkv = np.asarray(b_qkv, dtype=np.float32)
    W_proj = np.asarray(W_proj, dtype=np.float32)

    wq_t = np.ascontiguousarray(
        W_qkv.astype(ml_dtypes.bfloat16).reshape(KT, 128, NM, 128)
        .transpose(2, 1, 0, 3))
    wp_b = W_proj.astype(ml_dtypes.bfloat16)
    identb = np.vstack([np.eye(64), np.eye(64)]).astype(ml_dtypes.bfloat16)
    # causal mask for diagonal blocks (sigma-local storage): slot s holds
    # in-block position t = 16*(s%8) + s//8; visible iff t_k <= t_q.
    sl = np.arange(128)
    tloc = 16 * (sl % 8) + sl // 8
    triu = (tloc[:, None] <= tloc[None, :]).astype(np.float32) \
        .astype(ml_dtypes.bfloat16)
    with_bias = bool(np.any(b_qkv))
    bias_t = np.ascontiguousarray(b_qkv.reshape(NM, 128).T) if with_bias else None

    x_bf = x.astype(ml_dtypes.bfloat16)
    in_maps = []
    for c in range(N_CORES):
        b, qi = c // 4, c % 4
        rows = x_bf[b, ROWS * qi:ROWS * qi + ROWS, :]       # [512, 1024]
        # xt[hp][p, j, r] = rows[256*hp + r, 128*j + p]
        xtT = np.ascontiguousarray(
            rows.T.reshape(KT, 128, ROWS).transpose(1, 0, 2))  # [128, KT, 512]
        xt = np.ascontiguousarray(
            np.stack([xtT[:, :, 0:256], xtT[:, :, 256:512]]))  # [2,128,KT,256]
        m = {
            "xt": xt,
            "wqkv": wq_t,
            "wproj": np.ascontiguousarray(
                wp_b[256 * qi:256 * qi + 256, :].reshape(2, 128, E)),
            "identb": identb,
            "triu": triu,
        }
        if with_bias:
            m["bqkv"] = bias_t
        in_maps.append(m)
    return in_maps, with_bias


def kernel(x, W_qkv, b_qkv, W_proj, b_proj, _run_kwargs=None):
    in_maps, with_bias = make_in_maps(x, W_qkv, b_qkv, W_proj)
    nc = _get_program(with_bias)
    res = bass_utils.run_bass_kernel_spmd(
        nc, in_maps, core_ids=list(range(N_CORES)), **(_run_kwargs or {}))
    out = np.zeros((B, S, E), np.float32)
    for c in range(N_CORES):
        out[c // 4] += res.results[c]["out"]
    # undo sigma-local storage: device row (per 128-block) 8*(t%16)+t//16
    # holds position t
    t = np.arange(128)
    perm = 8 * (t % 16) + t // 16
    out = np.ascontiguousarray(
        out.reshape(B, S // 128, 128, E)[:, :, perm, :].reshape(B, S, E))
    out += np.asarray(b_proj, dtype=np.float32)[None, None, :]
    if _run_kwargs:
        kernel.last_results = res
    return out


# revision 23
# speedup vs baseline: 1.5305x; 1.0692x over previous
"""Trainium2 Bass kernel for causal self-attention (nn_Casualselfatt).

Reference computes (B=2, S=2048, E=1024, H=16, D=64, fp32):
    qkv = x @ W_qkv + b_qkv ; q,k,v = split(qkv)
    q = q.reshape(B, H, S, D)   # NOTE: raw reshape, no transpose.
    ...causal softmax attention per (b,h)...
    out = res @ W_proj + b_proj

The raw reshape means head h of batch b attends over the [S, D] reshape of
rows [128h, 128h+128) of q/k/v[b].  Sharding: 32 (b,h) pairs -> 4 heads of
one batch per core (core c: b=c//4, heads 4*(c%4)..+4).  Each core computes
a partial projection output; the host sums 4 partials per batch.

On-chip: scores are built transposed ([k-part, q-free]) so the softmax
denominator rides an appended ones-column through the AV matmul.  QKV runs
in bf16 (fp32 accumulate); scores run in float32r; the post-softmax path is
bf16.  x arrives pre-transposed from the host (bf16), W_qkv is SBUF-resident
(streamed once), and QKV psum->SBUF copies are pair-merged and split across
DVE/ACT.  PSUM: sc pool 4 banks + shared 1-bank work pool (qkv ps / av /
vtrans vp / proj pp) x4 bufs so attention groups double-buffer across g.
Softmax reciprocal uses the fast approx DVE op (~5x cheaper).
"""

import numpy as np
import ml_dtypes

import concourse.bass as bass
import concourse.tile as tile
from concourse import bacc, mybir
import concourse.bass_utils as bass_utils

F32 = mybir.dt.float32
F32R = mybir.dt.float32r
BF16 = mybir.dt.bfloat16

B, S, E = 2, 2048, 1024
H, D = 16, 64
N_CORES = 8
HEADS_PER_CORE = 4
ROWS = 128 * HEADS_PER_CORE  # x rows per core
NM = 24                      # qkv column chunks of 128 (q:0-7, k:8-15, v:16-23)
KT = 8                       # contraction tiles over E
NG = 4                       # q groups of 512
NB = S // 128                # 16 blocks of 128 along s'


def build_program(with_qkv_bias: bool):
    nc = bacc.Bacc("TRN2", target_bir_lowering=False, debug=False,
                   num_devices=N_CORES)

    # x^T per head-pair: [hp, 128 (e%128), KT, 256 (rows)] bf16, host-built
    xt_in = nc.dram_tensor("xt", [2, 128, KT, 256], BF16, kind="ExternalInput")
    wqkv = nc.dram_tensor("wqkv", [NM, 128, KT, 128], BF16, kind="ExternalInput")
    wproj = nc.dram_tensor("wproj", [2, 128, E], BF16, kind="ExternalInput")
    identb_in = nc.dram_tensor("identb", [128, 64], BF16, kind="ExternalInput")
    triu_in = nc.dram_tensor("triu", [128, 128], BF16, kind="ExternalInput")
    if with_qkv_bias:
        bqkv = nc.dram_tensor("bqkv", [128, NM], F32, kind="ExternalInput")
    out = nc.dram_tensor("out", [S, E], F32, kind="ExternalOutput")

    with tile.TileContext(nc) as tc:
        with (
            tc.tile_pool(name="const", bufs=1) as constp,
            tc.tile_pool(name="persist", bufs=1) as persist,
        ):
            # QKV-critical data first: x^T halves then W_qkv chunks.
            xTh = [persist.tile([128, KT, 256], BF16, tag=f"xT{i}",
                                name=f"xT{i}") for i in range(2)]
            nc.sync.dma_start(xTh[0][:, 0:4, :], xt_in.ap()[0][:, 0:4, :])
            nc.sync.dma_start(xTh[0][:, 4:8, :], xt_in.ap()[0][:, 4:8, :])
            # W_qkv SBUF-resident, streamed once (24 chunk DMAs) in the
            # order qkv_phase consumes them (V chunks first).
            wq_order = list(range(16, 24)) + list(range(16))
            wq_sb = persist.tile([128, NM, KT, 128], BF16, tag="wq")
            for m in wq_order[:4]:
                nc.sync.dma_start(wq_sb[:, m, :, :], wqkv.ap()[m])
            nc.sync.dma_start(xTh[1][:], xt_in.ap()[1])
            for m in wq_order[4:]:
                nc.sync.dma_start(wq_sb[:, m, :, :], wqkv.ap()[m])

            identb = constp.tile([128, 64], BF16)
            nc.sync.dma_start(identb[:], identb_in.ap())
            triu = constp.tile([128, 128], BF16)
            nc.sync.dma_start(triu[:], triu_in.ap())
            if with_qkv_bias:
                bias_sb = constp.tile([128, NM], F32)
                nc.sync.dma_start(bias_sb[:], bqkv.ap())

            wp_sb = [persist.tile([128, E], BF16, tag=f"wp{i}", name=f"wp{i}")
                     for i in range(2)]
            for i in range(2):
                nc.sync.dma_start(wp_sb[i][:], wproj.ap()[i])

            # Q/K transposed per head-pair: [128 (2 heads x 64 d), 2048 (s')]
            qt = [persist.tile([128, S], F32R, tag=f"qt{i}", name=f"qt{i}")
                  for i in range(2)]
            kt_ = [persist.tile([128, S], F32R, tag=f"kt{i}", name=f"ktt{i}")
                   for i in range(2)]
            # V^T per head [64 (d), 2048 (s')], later transposed into vnat
            vt = [persist.tile([64, S], BF16, tag=f"vt{i}", name=f"vt{i}")
                  for i in range(4)]
            # V natural per head: 16 blocks of [128, 65] (col 64 = ones)
            vnat = [persist.tile([128, NB * 65], BF16, tag=f"vn{i}",
                                 name=f"vn{i}") for i in range(4)]
            # res^T per head-pair (normalized), bf16
            res = [persist.tile([128, S], BF16, tag=f"res{i}", name=f"res{i}")
                   for i in range(2)]

            with (
                tc.tile_pool(name="wk", bufs=2, space="PSUM") as wk,
                tc.tile_pool(name="avps", bufs=2, space="PSUM") as avps,
                tc.tile_pool(name="scps", bufs=2, space="PSUM") as scps,
                tc.tile_pool(name="expp", bufs=8) as expp,
                tc.tile_pool(name="nrm", bufs=4) as nrm,
                tc.tile_pool(name="osb", bufs=3) as osb,
            ):
                def add_bias(dst_ap, src_ap, m, partn=64):
                    base = src_ap.base_partition()
                    nc.vector.tensor_scalar_add(
                        dst_ap, src_ap, bias_sb[base:base + partn, m:m + 1])

                def qkv_copy(dst_ap, src_ap, on_act):
                    if on_act:
                        nc.scalar.copy(dst_ap, src_ap)
                    else:
                        nc.vector.tensor_copy(dst_ap, src_ap)

                def qkv_phase(hp):
                    # m-chunk pairs; ps[128, 512] = 2 chunks x 256 rows.
                    # V pairs first so vtrans can overlap the q/k copy drain.
                    order = list(range(8, 12)) + list(range(8))
                    for mp in order:
                        m0 = 2 * mp
                        ps = wk.tile([128, 512], F32, tag="wk",
                                     name=f"ps{hp}_{mp}")
                        for c in range(2):
                            for j in range(KT):
                                nc.tensor.matmul(
                                    ps[:, 256 * c:256 * c + 256],
                                    wq_sb[:, m0 + c, j, :],
                                    xTh[hp][:, j, :],
                                    start=(j == 0), stop=(j == KT - 1))
                        ps4 = ps[:].rearrange("p (ch po jj rl) -> p ch po jj rl",
                                              ch=2, po=2, rl=8)
                        if with_qkv_bias:
                            # slow path: per-chunk bias adds (unmerged)
                            for c in range(2):
                                m = m0 + c
                                if m < 16:
                                    dstt = (qt if m < 8 else kt_)[hp]
                                    mm = m if m < 8 else m - 8
                                    dv = dstt[:].rearrange(
                                        "p (jj sb rl) -> p sb jj rl",
                                        sb=16, rl=8)
                                    for pos in range(2):
                                        rb = ps[:, 256 * c + 128 * pos:
                                                256 * c + 128 * pos + 128]
                                        rb3 = rb.rearrange(
                                            "p (jj rl) -> p jj rl", rl=8)
                                        add_bias(dv[64 * pos:64 * pos + 64,
                                                    2 * mm, :, :],
                                                 rb3[0:64, :, :], m)
                                        add_bias(dv[64 * pos:64 * pos + 64,
                                                    2 * mm + 1, :, :],
                                                 rb3[64:128, :, :], m)
                                else:
                                    mm = m - 16
                                    for pos in range(2):
                                        head = 2 * hp + pos
                                        dv = vt[head][:].rearrange(
                                            "p (jj sb rl) -> p sb jj rl",
                                            sb=16, rl=8)
                                        rb = ps[:, 256 * c + 128 * pos:
                                                256 * c + 128 * pos + 128]
                                        rb3 = rb.rearrange(
                                            "p (jj rl) -> p jj rl", rl=8)
                                        add_bias(dv[:, 2 * mm, :, :],
                                                 rb3[0:64, :, :], m)
                                        add_bias(dv[:, 2 * mm + 1, :, :],
                                                 rb3[64:128, :, :], m)
                            continue
                        # fast path: merged pair copies
                        # ACT only takes copies in phase 0 (idle there);
                        # during phase 1 the ACT is saturated with exp.
                        if m0 < 16:
                            dstt = (qt if m0 < 8 else kt_)[hp]
                            mm0 = m0 if m0 < 8 else m0 - 8
                            dvt = dstt[:].rearrange(
                                "p (jj cp two rl) -> p cp two jj rl",
                                cp=8, two=2, rl=8)
                            for pos in range(2):
                                for sub in range(2):
                                    src = ps4[64 * sub:64 * sub + 64, :,
                                              pos, :, :]
                                    dst = dvt[64 * pos:64 * pos + 64,
                                              mm0:mm0 + 2, sub, :, :]
                                    qkv_copy(dst, src, on_act=(pos == sub))
                        else:
                            mm0 = m0 - 16
                            for pos in range(2):
                                head = 2 * hp + pos
                                dvt = vt[head][:].rearrange(
                                    "p (jj cp two rl) -> p cp two jj rl",
                                    cp=8, two=2, rl=8)
                                for sub in range(2):
                                    src = ps4[64 * sub:64 * sub + 64, :,
                                              pos, :, :]
                                    dst = dvt[:, mm0:mm0 + 2, sub, :, :]
                                    qkv_copy(dst, src,
                                             on_act=(hp == 0 and sub == 0))

                def vtrans_phase(hp):
                    # V blocks: transpose [64, 128] slices to natural order,
                    # two blocks per psum tile, one merged copy out.
                    for pos in range(2):
                        head = 2 * hp + pos
                        for jj in range(0, NB, 2):
                            vp = wk.tile([128, 512], BF16, tag="wk",
                                         name=f"vp{head}_{jj}")
                            nc.tensor.transpose(
                                vp[:, 0:64],
                                vt[head][:, 128 * jj:128 * jj + 128],
                                identb[0:64, :])
                            nc.tensor.transpose(
                                vp[:, 64:128],
                                vt[head][:, 128 * (jj + 1):128 * (jj + 1) + 128],
                                identb[0:64, :])
                            src = vp[:, 0:128].rearrange(
                                "p (b d) -> p b d", b=2)
                            dst = vnat[head][:].rearrange(
                                "p (b c) -> p b c", c=65)[:, jj:jj + 2, 0:64]
                            if jj % 4 == 0:
                                nc.vector.tensor_copy(dst, src)
                            else:
                                nc.scalar.copy(dst, src)
                        nc.vector.memset(
                            vnat[head][:].rearrange(
                                "p (jj c) -> p jj c", c=65)[:, :, 64], 1.0)

                def attn_g(hp, g):
                    q0 = 512 * g
                    av = [avps.tile([65, 512], F32, tag="av",
                                    name=f"av{hp}_{g}_{i}") for i in range(2)]
                    nkb = 4 * g + 4
                    for kb in range(nkb):
                        ingroup = kb >= 4 * g
                        coff = 128 * (kb - 4 * g) if ingroup else 0
                        sc = scps.tile([128, 1024], F32, tag="sc")
                        ex = expp.tile([128, 1024], BF16, tag="ex")
                        for pos in range(2):
                            so = 512 * pos
                            nc.tensor.matmul(
                                sc[:, so + coff:so + 512],
                                kt_[hp][64 * pos:64 * pos + 64,
                                        128 * kb:128 * kb + 128],
                                qt[hp][64 * pos:64 * pos + 64,
                                       q0 + coff:q0 + 512],
                                start=True, stop=True,
                                tile_position=(64 * pos, 0))
                        if not ingroup:
                            nc.scalar.activation(
                                ex[:], sc[:],
                                mybir.ActivationFunctionType.Exp,
                                scale=0.125)
                        else:
                            sc3 = sc[:].rearrange("p (s q) -> p s q", s=2)
                            ex3 = ex[:].rearrange("p (s q) -> p s q", s=2)
                            nc.scalar.activation(
                                ex3[:, :, coff:512],
                                sc3[:, :, coff:512],
                                mybir.ActivationFunctionType.Exp,
                                scale=0.125)
                            for pos in range(2):
                                so = 512 * pos
                                nc.vector.tensor_mul(
                                    ex[:, so + coff:so + coff + 128],
                                    ex[:, so + coff:so + coff + 128],
                                    triu[:])
                        for pos in range(2):
                            so = 512 * pos
                            head = 2 * hp + pos
                            nc.tensor.matmul(
                                av[pos][:, coff:512],
                                vnat[head][:, 65 * kb:65 * kb + 65],
                                ex[:, so + coff:so + 512],
                                start=(kb == 0), stop=(kb == nkb - 1))
                    for pos in range(2):
                        # evacuate av -> SBUF promptly so the psum slot frees
                        den = nrm.tile([1, 512], F32, tag="den")
                        nc.vector.tensor_copy(den[:], av[pos][64:65, :])
                        avs = nrm.tile([64, 512], F32, tag="avs")
                        nc.vector.tensor_copy(avs[:], av[pos][0:64, :])
                        rec = nrm.tile([1, 512], F32, tag="rec")
                        nc.vector.reciprocal_approx_fast(
                            out=rec[:], in_=den[:])
                        bc = nrm.tile([64, 512], F32, tag="bc")
                        nc.gpsimd.partition_broadcast(bc[:], rec[:])
                        nc.vector.tensor_mul(
                            res[hp][64 * pos:64 * pos + 64, q0:q0 + 512],
                            avs[:], bc[:])

                def proj_group(gg):
                    for blk in range(4 * gg, 4 * gg + 4):
                        o = osb.tile([128, E], F32, name=f"o{blk}", tag="o")
                        for f in range(2):
                            pp = wk.tile([128, 512], F32, tag="wk",
                                         name=f"pp{blk}_{f}")
                            for hp in range(2):
                                nc.tensor.matmul(
                                    pp[:], res[hp][:, 128 * blk:128 * blk + 128],
                                    wp_sb[hp][:, 512 * f:512 * f + 512],
                                    start=(hp == 0), stop=(hp == 1))
                            if f == 0:
                                nc.vector.tensor_copy(o[:, 0:512], pp[:])
                            else:
                                nc.scalar.copy(o[:, 512:1024], pp[:])
                        nc.sync.dma_start(
                            out.ap()[128 * blk:128 * blk + 128, :], o[:])

                qkv_phase(0)
                vtrans_phase(0)
                for g in range(NG):
                    attn_g(0, g)
                qkv_phase(1)
                vtrans_phase(1)
                for g in range(NG):
                    attn_g(1, g)
                    proj_group(g)

    nc.compile()
    return nc


_CACHE = {}


def _get_program(with_qkv_bias: bool):
    if with_qkv_bias not in _CACHE:
        _CACHE[with_qkv_bias] = build_program(with_qkv_bias)
    return _CACHE[with_qkv_bias]


def make_in_maps(x, W_qkv, b_qkv, W_proj):
    """Build the 8 per-core input maps (host-side data marshaling only)."""
    x = np.ascontiguousarray(np.asarray(x, dtype=np.float32))
    W_qkv = np.asarray(W_qkv, dtype=np.float32)
    b_qkv = np.asarray(b_qkv, dtype=np.float32)
    W_proj = np.asarray(W_proj, dtype=np.float32)

    wq_t = np.ascontiguousarray(
        W_qkv.astype(ml_dtypes.bfloat16).reshape(KT, 128, NM, 128)
        .transpose(2, 1, 0, 3))
    wp_b = W_proj.astype(ml_dtypes.bfloat16)
    identb = np.vstack([np.eye(64), np.eye(64)]).astype(ml_dtypes.bfloat16)
    # causal mask for diagonal blocks (sigma-local storage): slot s holds
    # in-block position t = 16*(s%8) + s//8; visible iff t_k <= t_q.
    sl = np.arange(128)
    tloc = 16 * (sl % 8) + sl // 8
    triu = (tloc[:, None] <= tloc[None, :]).astype(np.float32) \
        .astype(ml_dtypes.bfloat16)
    with_bias = bool(np.any(b_qkv))
    bias_t = np.ascontiguousarray(b_qkv.reshape(NM, 128).T) if with_bias else None

    x_bf = x.astype(ml_dtypes.bfloat16)
    in_maps = []
    for c in range(N_CORES):
        b, qi = c // 4, c % 4
        rows = x_bf[b, ROWS * qi:ROWS * qi + ROWS, :]       # [512, 1024]
        # xt[hp][p, j, r] = rows[256*hp + r, 128*j + p]
        xtT = np.ascontiguousarray(
            rows.T.reshape(KT, 128, ROWS).transpose(1, 0, 2))  # [128, KT, 512]
        xt = np.ascontiguousarray(
            np.stack([xtT[:, :, 0:256], xtT[:, :, 256:512]]))  # [2,128,KT,256]
        m = {
            "xt": xt,
            "wqkv": wq_t,
            "wproj": np.ascontiguousarray(
                wp_b[256 * qi:256 * qi + 256, :].reshape(2, 128, E)),
            "identb": identb,
            "triu": triu,
        }
        if with_bias:
            m["bqkv"] = bias_t
        in_maps.append(m)
    return in_maps, with_bias


def kernel(x, W_qkv, b_qkv, W_proj, b_proj, _run_kwargs=None):
    in_maps, with_bias = make_in_maps(x, W_qkv, b_qkv, W_proj)
    nc = _get_program(with_bias)
    res = bass_utils.run_bass_kernel_spmd(
        nc, in_maps, core_ids=list(range(N_CORES)), **(_run_kwargs or {}))
    out = np.zeros((B, S, E), np.float32)
    for c in range(N_CORES):
        out[c // 4] += res.results[c]["out"]
    # undo sigma-local storage: device row (per 128-block) 8*(t%16)+t//16
    # holds position t
    t = np.arange(128)
    perm = 8 * (t % 16) + t // 16
    out = np.ascontiguousarray(
        out.reshape(B, S // 128, 128, E)[:, :, perm, :].reshape(B, S, E))
    out += np.asarray(b_proj, dtype=np.float32)[None, None, :]
    if _run_kwargs:
        kernel.last_results = res
    return out
